# revision 30
# baseline (speedup 1.0000x reference)
"""Causal multi-head attention (B=4, S=2048, D=1024, H=16) on 8 NeuronCores.

Sharding: core c handles batch b=c//2 and head-group g=c%2 (8 heads, 512
features). The host pre-transposes x and the weight slices (all bf16) so every
device matmul contracts along the partition dim; the row-parallel
out-projection partials (bf16) are summed pairwise on the host (+ bias).

Single fused pipeline per core (one Bass/Tile program, SPMD over 8 cores):
the QKV projections, causal flash attention, and out-projection are emitted
as ONE interleaved PE stream so the scalar-engine exp work (≈138us) hides
under the PE work (≈235us) instead of serializing behind it:
  - projections run as (m-tile, token-block) chains of 8 dk-accumulate
    matmuls; the token-block waves match the attention q-block waves, so
    attention for q-block qb starts as soon as wave nb=qb of q/k and v
    token-tiles <= 4qb+3 exist.
  - attention processes head pairs with 2-k-tile score groups pair-packed
    into one [128,2048] PSUM tile (4 banks) so one exp instruction covers
    both heads; diagonal groups split [j0,j1]/[j2,j3] with multiplicative
    triangular masks; context accumulates per head as ctx_aug[65,512] with a
    ones-column in V producing the softmax denominator in row 64.
  - leftover projection chains and the per-q-block normalize+out-project
    work are drained from a filler queue between attention groups, keeping
    the PE busy where the scores->exp->ctx chain would stall.
  - normalization: denominators staged via a p64 hop + row DMA, batched
    vector reciprocal per q-block, recip split hi/lo into bf16 and broadcast
    with K=2 PE outer products; one fused in-place multiply per (pair,
    q-block) on the bf16 unnormalized context.
"""

import sys
import types
from collections import OrderedDict

import numpy as np
import ml_dtypes

import concourse.bass as bass
import concourse.mybir as mybir
from concourse import tile
from concourse.bass_utils import run_bass_kernel_spmd
from concourse.masks import make_upper_triangular

# ----------------------------------------------------------------------------
# Compat patches for this container (self-contained on purpose).
# ----------------------------------------------------------------------------


def _patch_tail_drain():
    """This walrus build accepts only ONE sync-wait per sync-engine
    instruction; TileContext's tail drain may carry several. Split extras
    onto dedicated 1-wait nops."""
    from concourse.vector_clock import ScopedClock

    def _drain_and_barrier(self, tick_clock, wait_clock):
        nc = self.nc
        drain_inst = nc.sync.drain()
        wait_clock.add_sem_waits(
            drain_inst.ins, ScopedClock({None: tick_clock.global_clock})
        )
        si = drain_inst.ins.sync_info
        if si is not None and len(si.on_wait) > 1:
            waits = list(si.on_wait)
            drain_inst.ins.sync_info = mybir.SyncInfo(
                on_wait=waits[:1], on_update=list(si.on_update)
            )
            for w in waits[1:]:
                n = nc.sync.nop()
                n.ins.sync_info = mybir.SyncInfo(on_wait=[w], on_update=[])

        nc.all_engine_barrier()
        assert self.sems is not None
        popped = nc._tile_sem_poison_stack.pop()
        assert popped is self._sem_poison
        nc.clear_and_free_semaphores(list(self.sems.allocated().values()))
        nc.all_engine_barrier()

    tile.TileContext._drain_and_barrier = _drain_and_barrier


def _patch_profiling():
    """Provide the NTFF profile hook (image's antenv lacks axon_hooks) and
    disable cloud artifact uploads. Only matters when tracing is requested."""
    import concourse.bass_utils as bass_utils

    bass_utils.upload_artifacts = lambda tmpdir: tmpdir
    try:
        from antenv.axon_hooks import get_axon_ntff_profile_hook  # noqa: F401
        return
    except ImportError:
        pass
    try:
        from trn_agent_boot.trn_boot import _ntff_profile_via_ctypes

        hook = _ntff_profile_via_ctypes("/opt/axon/libaxon_pjrt.so")
    except Exception:
        hook = None
    mod = types.ModuleType("antenv.axon_hooks")
    mod._hook = hook
    mod.get_axon_ntff_profile_hook = lambda: mod._hook
    mod.set_axon_ntff_profile_hook = lambda h: setattr(mod, "_hook", h)
    sys.modules["antenv.axon_hooks"] = mod
    import antenv

    antenv.axon_hooks = mod


_patch_tail_drain()
_patch_profiling()


def _legalize_waits(nc):
    """This walrus build allows 1 sync-wait per instruction (2 on
    EventSemaphore). Split excess waits onto EventSemaphore carriers
    inserted just before the over-capacity instruction (same engine
    queue, so ordering semantics are preserved)."""
    n_fix = 0
    for f in nc.m.functions:
        for b in f.blocks:
            out = []
            changed = False
            for inst in b.instructions:
                si = inst.sync_info
                cap = 1
                if si is not None and len(si.on_wait) > cap:
                    waits = list(si.on_wait)
                    extra, keep = waits[:-cap], waits[-cap:]
                    for i in range(0, len(extra), 1):
                        n_fix += 1
                        out.append(
                            mybir.InstNoOp(
                                name=f"I-waitfix-{n_fix}",
                                engine=inst.engine,
                                ins=[],
                                outs=[],
                                sync_info=mybir.SyncInfo(
                                    on_wait=extra[i:i + 1], on_update=[]
                                ),
                            )
                        )
                    inst.sync_info = mybir.SyncInfo(
                        on_wait=keep, on_update=list(si.on_update)
                    )
                    changed = True
                out.append(inst)
            if changed:
                b.instructions = out

# ----------------------------------------------------------------------------
# Problem constants (hardcoded; kernel.py must be self-contained).
# ----------------------------------------------------------------------------
B, S, D, H = 4, 2048, 1024, 16
HD = D // H          # 64 head dim
NCORES = 8
GPC = 2              # head-groups per batch (cores per batch)
FPC = D // GPC       # 512 features per core
HPC = H // GPC       # 8 heads per core
P = 128
DC = D // P          # 8 contraction chunks
NT = S // P          # 16 token tiles
QB = 512             # q-block
NQB = S // QB        # 4

F32 = mybir.dt.float32
BF16 = mybir.dt.bfloat16
EXPF = mybir.ActivationFunctionType.Exp
SCALE = 1.0 / np.sqrt(HD)


def _build_program():
    nc = bass.Bass("TRN2", target_bir_lowering=False, debug=False, num_devices=1)
    xT = nc.dram_tensor("xT", [D, S], BF16, kind="ExternalInput").ap()
    wq = nc.dram_tensor("wq", [D, FPC], BF16, kind="ExternalInput").ap()
    wk = nc.dram_tensor("wk", [D, FPC], BF16, kind="ExternalInput").ap()
    wv = nc.dram_tensor("wv", [D, FPC], BF16, kind="ExternalInput").ap()
    wo = nc.dram_tensor("wo", [FPC, D], BF16, kind="ExternalInput").ap()
    out = nc.dram_tensor("out", [S, D], BF16, kind="ExternalOutput").ap()

    with tile.TileContext(nc) as tc:
        _emit(nc, tc, xT, wq, wk, wv, wo, out)
    _legalize_waits(nc)
    return nc


def _emit(nc, tc, xT, wq, wk, wv, wo, out):
    persist = tc.alloc_tile_pool(name="persist", bufs=1)

    xT_sb = persist.tile([P, DC, S], BF16, tag="xT_sb")
    wq_sb = persist.tile([P, DC, FPC], BF16, tag="wq_sb")
    wk_sb = persist.tile([P, DC, FPC], BF16, tag="wk_sb")
    wv_sb = persist.tile([P, DC, FPC], BF16, tag="wv_sb")
    wo_sb = persist.tile([P, FPC // P, D], BF16, tag="wo_sb")
    qT = persist.tile([P, NQB, S], BF16, tag="qT")
    kT = persist.tile([P, NQB, S], BF16, tag="kT")
    vtm = persist.tile([P, NT, HPC, HD + 1], BF16, tag="vtm")
    ctxU = persist.tile([P, NQB, S], BF16, tag="ctxU")
    dmask_f = persist.tile([P, P], F32, tag="dmask_f")
    dmask = persist.tile([P, P], BF16, tag="dmask")
    denstage = persist.tile([P, QB], F32, tag="denstage")  # row qb*32+h
    recstage = persist.tile([P, QB], F32, tag="recstage")
    recbf = persist.tile([P, QB], BF16, tag="recbf")
    # q-block 3's k-tiles 0..7 run early (while ACT is idle); partial ctx
    # saved here (f32) and re-loaded into PSUM for the second half.
    ctxSav = persist.tile([P, HPC, QB], F32, tag="ctxSav")
    onesbf = persist.tile([P, HD], BF16, tag="onesbf")
    # bf16 recip rows for the K=1 broadcast outer product; reused across
    # qb parity (consumed by C(qb) during B(qb+1)).
    rrowbig = persist.tile([P, 2 * HPC, QB], BF16, tag="rrowbig")

    # ---------------- DMA schedule (priority order) ----------------
    for dk in range(DC):
        nc.sync.dma_start(wq_sb[:, dk, :], wq[dk * P:(dk + 1) * P, :])
        nc.sync.dma_start(xT_sb[:, dk, 0:QB], xT[dk * P:(dk + 1) * P, 0:QB])
    for dk in range(DC):
        nc.sync.dma_start(wk_sb[:, dk, :], wk[dk * P:(dk + 1) * P, :])
    for dk in range(DC):
        nc.sync.dma_start(wv_sb[:, dk, :], wv[dk * P:(dk + 1) * P, :])
    for nb in range(1, NQB):
        for dk in range(DC):
            nc.sync.dma_start(
                xT_sb[:, dk, nb * QB:(nb + 1) * QB],
                xT[dk * P:(dk + 1) * P, nb * QB:(nb + 1) * QB],
            )
        if nb == 1:
            nc.sync.dma_start(wo_sb[:], wo.rearrange("(c p) e -> p c e", p=P))

    # one-time setup
    make_upper_triangular(nc, dmask_f[:], val=1.0, diag=True)
    nc.vector.tensor_copy(dmask[:], dmask_f[:])
    nc.vector.memset(vtm[:, :, :, HD:HD + 1], 1.0)  # ones column only
    nc.vector.memset(onesbf[:], 1.0)

    OFFS = (0, 512, 1024, 1280)
    LENS = (512, 384, 256, 128)
    QOFFS = (0, 128, 256, 384)

    fillp = tc.alloc_tile_pool(name="fillp", bufs=2, space="PSUM")   # 2 banks
    scps = tc.alloc_tile_pool(name="scps", bufs=1, space="PSUM")     # 4 banks
    ctxps = tc.alloc_tile_pool(name="ctxps", bufs=1, space="PSUM")   # 2 banks
    expp = tc.alloc_tile_pool(name="expp", bufs=4)
    dtmpp = tc.alloc_tile_pool(name="dtmpp", bufs=3)
    outsb = tc.alloc_tile_pool(name="outsb", bufs=3)

    # ---------------- filler units (projection chains, C units) -----------
    def qk_chain(w_sb, dst, m, nb):
        def emit():
            ps = fillp.tile([P, QB], F32, tag="fill")
            for dk in range(DC):
                nc.tensor.matmul(
                    ps[:],
                    lhsT=w_sb[:, dk, m * P:(m + 1) * P],
                    rhs=xT_sb[:, dk, nb * QB:(nb + 1) * QB],
                    start=(dk == 0),
                    stop=(dk == DC - 1),
                    skip_group_check=True,
                )
            nc.vector.tensor_copy(dst[:, m, nb * QB:(nb + 1) * QB], ps[:])
        return emit

    def v_chain(nt):
        def emit():
            ps = fillp.tile([P, FPC], F32, tag="fill")
            for dk in range(DC):
                nc.tensor.matmul(
                    ps[:],
                    lhsT=xT_sb[:, dk, nt * P:(nt + 1) * P],
                    rhs=wv_sb[:, dk, :],
                    start=(dk == 0),
                    stop=(dk == DC - 1),
                    skip_group_check=True,
                )
            nc.vector.tensor_copy(
                vtm[:, nt, :, 0:HD],
                ps[:].rearrange("p (h d) -> p h d", h=HPC),
            )
        return emit

    def c_norm_unit(qb, m2):
        def emit():
            bcps = fillp.tile([P, QB], F32, tag="fill")
            for half in range(2):
                ridx = (qb % 2) * HPC + 2 * m2 + half
                nc.tensor.matmul(
                    bcps[half * HD:(half + 1) * HD, :],
                    lhsT=onesbf[HD:HD + 1, :],
                    rhs=rrowbig[HD:HD + 1, ridx, :],
                    start=True,
                    stop=True,
                    skip_group_check=True,
                )
            sl = ctxU[:, m2, qb * QB:(qb + 1) * QB]
            nc.vector.tensor_mul(sl, sl, bcps[:])
        return emit

    def c_out_unit(nt):
        def emit():
            stage = outsb.tile([P, D], BF16, tag="stage")
            for ec in range(D // QB):
                pso = fillp.tile([P, QB], F32, tag="fill", name=f"pso{ec}")
                for m in range(FPC // P):
                    nc.tensor.matmul(
                        pso[:],
                        lhsT=ctxU[:, m, nt * P:(nt + 1) * P],
                        rhs=wo_sb[:, m, ec * QB:(ec + 1) * QB],
                        start=(m == 0),
                        stop=(m == FPC // P - 1),
                        skip_group_check=True,
                    )
                nc.vector.tensor_copy(stage[:, ec * QB:(ec + 1) * QB], pso[:])
                nc.sync.dma_start(
                    out[nt * P:(nt + 1) * P, ec * QB:(ec + 1) * QB],
                    stage[:, ec * QB:(ec + 1) * QB],
                )
        return emit

    fillers = OrderedDict()

    def run_unit(key):
        fn = fillers.pop(key, None)
        if fn is not None:
            fn()

    def drain(n=1, reserve=0):
        for _ in range(n):
            if len(fillers) <= reserve:
                return
            fillers.pop(next(iter(fillers)))()

    # ---------------- upfront wave-0 head-pair 0 ----------------
    qk_chain(wq_sb, qT, 0, 0)()
    qk_chain(wk_sb, kT, 0, 0)()
    for nt in range(4):
        v_chain(nt)()

    for m in range(NQB):
        for nb in range(NQB):
            if not (m == 0 and nb == 0):
                fillers[("q", m, nb)] = qk_chain(wq_sb, qT, m, nb)
                fillers[("k", m, nb)] = qk_chain(wk_sb, kT, m, nb)
        if m >= 1:
            for nt in range(4 * m, 4 * m + 4):
                fillers[("v", nt)] = v_chain(nt)
    fillers = OrderedDict(
        sorted(fillers.items(), key=lambda kv: (kv[0][2] if kv[0][0] in "qk"
                                                else kv[0][1] // 4, kv[0][0]))
    )

    # ---------------- fused attention + fillers ----------------
    # Full score groups: 2 k-tiles, both heads packed in one [P, 2048] PSUM
    # tile (head a cols 0:1024, head b cols 1024:2048) -> single exp.
    # Diag groups: G1=[j0,j1] (a 0:896, b 896:1792), G2=[j2,j3] (a 0:384,
    # b 384:768).
    def emit_scores_exp(m2, qb, kind, kts):
        sc = scps.tile([P, 4 * QB], F32, tag="sc")
        if kind == "full":
            w = len(kts) * QB
            es = expp.tile([P, 4 * QB], BF16, tag="es")
            for hh in (2 * m2, 2 * m2 + 1):
                hp = (hh % 2) * HD
                base = (hh % 2) * w
                for i, kt in enumerate(kts):
                    nc.tensor.matmul(
                        sc[:, base + i * QB:base + (i + 1) * QB],
                        lhsT=kT[hp:hp + HD, m2, kt * P:(kt + 1) * P],
                        rhs=qT[hp:hp + HD, m2, qb * QB:(qb + 1) * QB],
                        start=True,
                        stop=True,
                    )
            nc.scalar.activation(es[:, 0:2 * w], sc[:, 0:2 * w], EXPF,
                                 scale=SCALE)
            return es
        # Diag segments must not cross 2KB PSUM banks: place each j at a
        # bank-aligned offset per head (head-b base = 2 banks up), and exp
        # each head's contiguous span separately.
        js = kts  # diag j indices
        segw = sum(LENS[j] for j in js)       # 896 (G1) or 384 (G2)
        hbase = 1024 if js[0] == 0 else 512   # head-b base offset
        es = expp.tile([P, 4 * QB], BF16, tag="es")
        for hh in (2 * m2, 2 * m2 + 1):
            hp = (hh % 2) * HD
            base = (hh % 2) * hbase
            o = 0
            for j in js:
                kt = 4 * qb + j
                nc.tensor.matmul(
                    sc[:, base + o:base + o + LENS[j]],
                    lhsT=kT[hp:hp + HD, m2, kt * P:(kt + 1) * P],
                    rhs=qT[hp:hp + HD, m2,
                           qb * QB + QOFFS[j]:(qb + 1) * QB],
                    start=(j != 3),
                    stop=(j != 2),
                    skip_group_check=True,
                )
                o += LENS[j]
        for hh in (2 * m2, 2 * m2 + 1):
            base = (hh % 2) * hbase
            nc.scalar.activation(
                es[:, base:base + segw], sc[:, base:base + segw], EXPF,
                scale=SCALE,
            )
            o = 0
            for j in js:
                nc.vector.tensor_mul(
                    es[:, base + o:base + o + P],
                    es[:, base + o:base + o + P],
                    dmask[:],
                )
                o += LENS[j]
        return es

    def make_ctx(m2, qb, kind, kts, es, pctx, first_ctx, mode):
        def emit():
            for hh in (2 * m2, 2 * m2 + 1):
                if kind == "full":
                    w = len(kts) * QB
                    base = (hh % 2) * w
                    for i, kt in enumerate(kts):
                        nc.tensor.matmul(
                            pctx[hh][0:HD + 1, :],
                            lhsT=vtm[:, kt, hh, :],
                            rhs=es[:, base + i * QB:base + (i + 1) * QB],
                            start=first_ctx[hh],
                            stop=False,
                            skip_group_check=True,
                        )
                        first_ctx[hh] = False
                else:
                    js = kts
                    base = (hh % 2) * (1024 if js[0] == 0 else 512)
                    o = 0
                    for j in js:
                        nc.tensor.matmul(
                            pctx[hh][0:HD + 1, QOFFS[j]:QB],
                            lhsT=vtm[:, 4 * qb + j, hh, :],
                            rhs=es[:, base + o:base + o + LENS[j]],
                            start=first_ctx[hh],
                            stop=(j == 3),
                            skip_group_check=True,
                        )
                        first_ctx[hh] = False
                        o += LENS[j]
            if mode == "mid":
                return
            if mode == "save":
                for hh in (2 * m2, 2 * m2 + 1):
                    slot = m2 * 2 + (hh % 2)
                    nc.vector.tensor_copy(
                        ctxSav[0:HD + 1, slot, :], pctx[hh][0:HD + 1, :]
                    )
                return
            for hh in (2 * m2, 2 * m2 + 1):
                hp = (hh % 2) * HD
                nc.vector.tensor_copy(
                    ctxU[hp:hp + HD, m2, qb * QB:(qb + 1) * QB],
                    pctx[hh][0:HD, :],
                )
                dtmp = dtmpp.tile([P, QB], F32, tag="dtmp")
                nc.vector.tensor_copy(
                    dtmp[HD:HD + 1, :], pctx[hh][HD:HD + 1, :]
                )
                idx = qb * 32 + hh
                nc.sync.dma_start(denstage[idx:idx + 1, :], dtmp[HD:HD + 1, :])
            if m2 == HPC // 2 - 1:
                # batched per-qb reciprocal + single bf16 broadcast row per
                # head (no hi/lo split; bf16 recip error ~0.4% is in budget)
                r = slice(qb * 32, qb * 32 + HPC)
                rc = slice((qb % 2) * HPC, (qb % 2) * HPC + HPC)
                nc.vector.reciprocal(recstage[r, :], denstage[r, :])
                nc.vector.tensor_copy(recbf[r, :], recstage[r, :])
                nc.sync.dma_start(rrowbig[HD:HD + 1, rc, :], recbf[r, :])
                for mm in range(NQB):
                    fillers[("cn", qb, mm)] = c_norm_unit(qb, mm)
        return emit

    pending = None
    H1KT = 8  # qb3 k-tiles 0..H1KT-1 processed early

    def b_block(qb, m2, part):
        nonlocal pending
        heads = (2 * m2, 2 * m2 + 1)
        run_unit(("q", m2, qb))
        kmax = H1KT // 4 - 1 if part == "h1" else qb
        for nb in range(kmax + 1):
            run_unit(("k", m2, nb))
        vmax = H1KT - 1 if part == "h1" else 4 * qb + 3
        for nt in range(vmax + 1):
            run_unit(("v", nt))

        pctx = {
            hh: ctxps.tile([P, QB], F32, tag=f"pctx{i}", name=f"pctx{i}")
            for i, hh in enumerate(heads)
        }
        if part == "h2":
            first_ctx = {hh: False for hh in heads}
            for hh in heads:
                slot = m2 * 2 + (hh % 2)
                nc.vector.tensor_copy(
                    pctx[hh][0:HD + 1, :], ctxSav[0:HD + 1, slot, :]
                )
        else:
            first_ctx = {hh: True for hh in heads}
        kt0 = H1KT if part == "h2" else 0
        kt1 = H1KT if part == "h1" else 4 * qb
        groups = [("full", [kt, kt + 1]) for kt in range(kt0, kt1, 2)]
        if part != "h1":
            groups.append(("diag", [0, 1]))
            groups.append(("diag", [2, 3]))
        for gi, (kind, kts) in enumerate(groups):
            es = emit_scores_exp(m2, qb, kind, kts)
            if pending is not None:
                pending()
            drain(1, reserve=2)
            if gi == len(groups) - 1:
                mode = "save" if part == "h1" else "final"
            else:
                mode = "mid"
            pending = make_ctx(m2, qb, kind, kts, es, pctx, first_ctx, mode)

    def finish_qb(qb):
        nonlocal pending
        pending()
        pending = None
        # cn units were queued per pair inside make_ctx; queue out-proj now
        for nt in range(4 * qb, 4 * qb + 4):
            fillers[("co", nt)] = c_out_unit(nt)

    for m2 in range(HPC // 2):
        b_block(0, m2, "all")
    finish_qb(0)
    for m2 in range(HPC // 2):
        b_block(3, m2, "h1")
    for qb in (1, 2):
        for m2 in range(HPC // 2):
            b_block(qb, m2, "all")
        finish_qb(qb)
    for m2 in range(HPC // 2):
        b_block(3, m2, "h2")
    finish_qb(3)

    while fillers:
        drain(1)

    outsb.release()
    dtmpp.release()
    expp.release()
    ctxps.release()
    scps.release()
    fillp.release()
    persist.release()


_program_cache = None
last_results = None


def _get_program():
    global _program_cache
    if _program_cache is None:
        _program_cache = _build_program()
    return _program_cache


def kernel(x, Wq, Wk, Wv, Wo, bo):
    global last_results
    x = np.asarray(x, dtype=np.float32)
    Wq = np.asarray(Wq, dtype=np.float32)
    Wk = np.asarray(Wk, dtype=np.float32)
    Wv = np.asarray(Wv, dtype=np.float32)
    Wo = np.asarray(Wo, dtype=np.float32)
    bo = np.asarray(bo, dtype=np.float32)

    bf = ml_dtypes.bfloat16
    in_maps = []
    for c in range(NCORES):
        b, g = c // GPC, c % GPC
        fs = slice(g * FPC, (g + 1) * FPC)
        in_maps.append(
            {
                "xT": np.ascontiguousarray(x[b].T).astype(bf),
                "wq": np.ascontiguousarray(Wq[fs, :].T).astype(bf),
                "wk": np.ascontiguousarray(Wk[fs, :].T).astype(bf),
                "wv": np.ascontiguousarray(Wv[fs, :].T).astype(bf),
                "wo": np.ascontiguousarray(Wo[:, fs].T).astype(bf),
            }
        )

    nc = _get_program()
    res = run_bass_kernel_spmd(nc, in_maps, core_ids=list(range(NCORES)))
    last_results = res

    outf = np.empty((B, S, D), dtype=np.float32)
    for b in range(B):
        outf[b] = (
            res.results[GPC * b]["out"].astype(np.float32)
            + res.results[GPC * b + 1]["out"].astype(np.float32)
            + bo
        )
    return outf
